# revision 13
# baseline (speedup 1.0000x reference)
"""Trainium2 Bass kernel for a 2-layer GAT (ConceptGAT), 8-core SPMD.

Sharding: destination-node parallel. Core r owns dst nodes
[r*2500, (r+1)*2500). Host routes each edge (incl. self-loops) to the core
owning its dst, sorts by dst, and tiles edges into 128-node dst windows
(K 128-edge tiles per window, padded with dstloc=-1 null edges).

Per core, on device:
  phase 0: build W1ext=[W1|wsrc|wdst] (wsrc[:,h]=sum_c W1[:,h*256+c]*as1[h,c])
           and W2ext=[W2|w2src|w2dst] in SBUF.
  phase 1: xp1ext = xT_local.T @ W1ext per 128-node window ->
           [xp1|a_src] rows to DRAM chunk (local 2500 rows), a_dst kept in
           SBUF. AllGather chunks -> table1 [20000, 832].
  phase 2: per window, dma_gather rows table1[src[e]] for the window's
           edges; per 128-edge tile build the one-hot matrix
           eqP[e,j] = (dstloc[e]==j) with a DVE is_equal against an iota
           constant; a_dst per edge via PE-transpose(eqP) matmul; attention
           ex = exp(lrelu(a_src+a_dst)); rhs = [ex_h * xp_h | ex]; aggregate
           S += eqP.T @ rhs in PSUM across the window's tiles.
  phase 3: epilogue per window: out = S_feat / S_ex per head + b1, ELU ->
           h; hT via PE transpose; xp2ext = hT.T @ W2ext -> chunk2;
           AllGather -> table2 [20000, 320].
  phase 4: same aggregation with table2 (1 head), + b2 -> out rows.

Host gathers the 8 output chunks. All host work is layout/indexing only
(shard, sort, pad, transpose, replicate); every FLOP runs on device.
"""

import os
import sys

import numpy as np

sys.path.insert(0, "/opt/trn_rl_repo")

from contextlib import ExitStack

import concourse.bacc as bacc
import concourse.tile as tile
from concourse import bass, mybir
from concourse.bass_utils import run_bass_kernel_spmd
from concourse.library_config import mlp
from concourse.masks import make_identity

F32 = mybir.dt.float32
I16 = mybir.dt.int16
AF = mybir.ActivationFunctionType
OP = mybir.AluOpType

R, C, P = 8, 2500, 128          # cores, dst-nodes/core, partition
NW = (C + P - 1) // P           # 20 windows per core
CP = NW * P                     # 2560 padded local nodes
N, IND, HID, H1 = 20000, 768, 256, 3
NEG = 0.2                       # leaky-relu slope (PyG GATConv default)
ROW1 = 832                      # table1 row stride, f32 (771 used; %64==0)
ROW2 = 320                      # table2 row stride, f32 (257 used)
NK1 = IND // P                  # 6 contraction chunks (768)


def cdiv(a, b):
    return (a + b - 1) // b


# ---------------------------------------------------------------- host prep

def prep_edges(edge_index):
    """Shard edges by dst core, sort by dst, window + tile + pad.

    Returns idx16 [R,NW,128,K*8] int16 (dma_gather wrapped layout),
    dstlocT [R,NW,128,K] f32 (edge-partition-major, -1 = null edge),
    dstloc [R,NW,K,128] f32, K.
    """
    src = np.concatenate([edge_index[0], np.arange(N)]).astype(np.int64)
    dst = np.concatenate([edge_index[1], np.arange(N)]).astype(np.int64)
    per_core = []
    for r in range(R):
        m = (dst >= r * C) & (dst < (r + 1) * C)
        s, d = src[m], dst[m] - r * C
        o = np.argsort(d, kind="stable")
        s, d = s[o], d[o]
        wins = []
        for w in range(NW):
            mw = (d >= w * P) & (d < (w + 1) * P)
            wins.append((s[mw], d[mw] - w * P))
        per_core.append(wins)
    K = max(max(cdiv(len(sw), P), 1)
            for core in per_core for (sw, _) in core)
    idx16 = np.zeros((R, NW, 128, K * P // 16), np.int16)
    dstloc = np.full((R, NW, K, P), -1.0, np.float32)
    for r in range(R):
        for w in range(NW):
            sw, dw = per_core[r][w]
            n = len(sw)
            sp = np.zeros(K * P, np.int64)
            sp[:n] = sw
            dl = np.full(K * P, -1.0, np.float32)
            dl[:n] = dw.astype(np.float32)
            dstloc[r, w] = dl.reshape(K, P)
            wrapped = sp.astype(np.int16).reshape(K * P // 16, 16).T
            idx16[r, w] = np.tile(wrapped, (8, 1))
    dstlocT = np.ascontiguousarray(dstloc.transpose(0, 1, 3, 2))
    return idx16, dstlocT, dstloc, K


# ------------------------------------------------------------ device program

def build_program(K):
    PH = int(os.environ.get("GAT_PHASES", "9"))
    NWL = int(os.environ.get("GAT_NWIN", str(NW)))
    nc = bacc.Bacc("TRN2", target_bir_lowering=False, debug=False,
                   num_devices=R, enable_asserts=False)

    def din(name, shape, dt=F32):
        return nc.dram_tensor(name, shape, dt, kind="ExternalInput")

    xT = din("xT", [IND, CP])                       # x_local.T, cols 2500+ zero
    W1d = din("W1", [IND, H1 * HID])
    W2d = din("W2", [H1 * HID, HID])
    b1bc = din("b1bc", [P, H1 * HID])
    b2bc = din("b2bc", [P, HID])
    as1bc = din("as1bc", [P, H1 * HID])
    ad1bc = din("ad1bc", [P, H1 * HID])
    as2bc = din("as2bc", [P, HID])
    ad2bc = din("ad2bc", [P, HID])
    iotaP = din("iotaP", [P, P])                    # each partition: 0..127
    idx16d = din("idx16", [NW, 128, K * 8], I16)
    dlocTd = din("dstlocT", [NW, 128, K])
    outd = nc.dram_tensor("out_local", [CP, HID], F32, kind="ExternalOutput")

    chunk1 = nc.dram_tensor("chunk1", [C, ROW1], F32)
    table1 = nc.dram_tensor("table1", [N, ROW1], F32, addr_space="Shared")
    chunk2 = nc.dram_tensor("chunk2", [C, ROW2], F32)
    table2 = nc.dram_tensor("table2", [N, ROW2], F32, addr_space="Shared")

    GRPS = [list(range(R))]
    G1 = cdiv(K, 2)              # gather split sizes (SBUF pressure)
    G2 = K - G1

    with tile.TileContext(nc) as tc, ExitStack() as ctx:
        nc.gpsimd.load_library(mlp)

        const = ctx.enter_context(tc.tile_pool(name="const", bufs=1))
        wext = ctx.enter_context(tc.tile_pool(name="wext", bufs=1))
        stage = ctx.enter_context(tc.tile_pool(name="stage", bufs=2))
        edgep = ctx.enter_context(tc.tile_pool(name="edgep", bufs=3))
        # two alternating tags (first/second half) give double-buffering
        gat = ctx.enter_context(tc.tile_pool(name="gat", bufs=1))
        xtp = ctx.enter_context(tc.tile_pool(name="xtp", bufs=3))
        ps_acc = ctx.enter_context(
            tc.tile_pool(name="ps_acc", bufs=2, space="PSUM"))
        ps_sm = ctx.enter_context(
            tc.tile_pool(name="ps_sm", bufs=1, space="PSUM"))
        ps_epi = ctx.enter_context(
            tc.tile_pool(name="ps_epi", bufs=1, space="PSUM"))

        # ---- constants
        iotaP_sb = const.tile([P, P], F32)
        nc.sync.dma_start(iotaP_sb[:], iotaP[:])
        ident = const.tile([P, P], F32)
        make_identity(nc, ident[:])
        b1_sb = const.tile([P, H1 * HID], F32)
        nc.sync.dma_start(b1_sb[:], b1bc[:])
        b2_sb = const.tile([P, HID], F32)
        nc.sync.dma_start(b2_sb[:], b2bc[:])
        as1_sb = const.tile([P, H1 * HID], F32)
        nc.sync.dma_start(as1_sb[:], as1bc[:])
        ad1_sb = const.tile([P, H1 * HID], F32)
        nc.sync.dma_start(ad1_sb[:], ad1bc[:])
        as2_sb = const.tile([P, HID], F32)
        nc.sync.dma_start(as2_sb[:], as2bc[:])
        ad2_sb = const.tile([P, HID], F32)
        nc.sync.dma_start(ad2_sb[:], ad2bc[:])
        adst1_sb = const.tile([P, NW * H1], F32)   # layer-1 a_dst, local
        a2dst_sb = const.tile([P, NW], F32)        # layer-2 a_dst, local
        trash = const.tile([P, HID], F32)

        # ---- phase 0: W1ext / W2ext
        w1e = []
        for c in range(NK1):
            t = wext.tile([P, 774], F32, tag=f"w1e{c}")
            nc.sync.dma_start(t[:, 0:768], W1d[c * P:(c + 1) * P, :])
            for h in range(H1):
                hs = slice(h * HID, (h + 1) * HID)
                nc.vector.tensor_mul(out=trash[:], in0=t[:, hs],
                                     in1=as1_sb[:, hs])
                nc.vector.tensor_reduce(
                    out=t[:, 768 + h:769 + h], in_=trash[:],
                    axis=mybir.AxisListType.X, op=OP.add)
                nc.vector.tensor_mul(out=trash[:], in0=t[:, hs],
                                     in1=ad1_sb[:, hs])
                nc.vector.tensor_reduce(
                    out=t[:, 771 + h:772 + h], in_=trash[:],
                    axis=mybir.AxisListType.X, op=OP.add)
            w1e.append(t)
        w2e = []
        for c in range(NK1):
            t = wext.tile([P, 258], F32, tag=f"w2e{c}")
            nc.sync.dma_start(t[:, 0:256], W2d[c * P:(c + 1) * P, :])
            nc.vector.tensor_mul(out=trash[:, 0:256], in0=t[:, 0:256],
                                 in1=as2_sb[:])
            nc.vector.tensor_reduce(
                out=t[:, 256:257], in_=trash[:, 0:256],
                axis=mybir.AxisListType.X, op=OP.add)
            nc.vector.tensor_mul(out=trash[:, 0:256], in0=t[:, 0:256],
                                 in1=ad2_sb[:])
            nc.vector.tensor_reduce(
                out=t[:, 257:258], in_=trash[:, 0:256],
                axis=mybir.AxisListType.X, op=OP.add)
            w2e.append(t)

        # ---- phase 1: xp1ext windows -> chunk1 + adst1_sb
        chunk1_writes = []
        for w in range(NW if PH >= 1 else 0):
            ps = ps_acc.tile([P, 774], F32, tag="acc")
            for c in range(NK1):
                xt_t = xtp.tile([P, P], F32, tag="xt")
                nc.sync.dma_start(
                    xt_t[:], xT[c * P:(c + 1) * P, w * P:(w + 1) * P])
                nc.tensor.matmul(out=ps[:, 0:512], lhsT=xt_t[:],
                                 rhs=w1e[c][:, 0:512],
                                 start=(c == 0), stop=(c == NK1 - 1))
                nc.tensor.matmul(out=ps[:, 512:774], lhsT=xt_t[:],
                                 rhs=w1e[c][:, 512:774],
                                 start=(c == 0), stop=(c == NK1 - 1))
            xp1_sb = stage.tile([P, 771], F32, tag="xp1")
            nc.vector.tensor_copy(out=xp1_sb[:], in_=ps[:, 0:771])
            nc.scalar.activation(out=adst1_sb[:, w * H1:(w + 1) * H1],
                                 in_=ps[:, 771:774], func=AF.Copy)
            rows = min(C - w * P, P)
            d = nc.sync.dma_start(out=chunk1[w * P:w * P + rows, 0:771],
                                  in_=xp1_sb[:rows, :])
            chunk1_writes.append(d)

        ag1 = None
        if PH >= 2:
            ag1 = nc.gpsimd.collective_compute(
                "AllGather", OP.bypass, replica_groups=GRPS,
                ins=[chunk1[:]], outs=[table1[:]])
            for d in chunk1_writes:
                tile.add_dep_helper(ag1.ins, d.ins, reason="chunk1 before AG1")

        # ---- edge-tile machinery (shared by both layers) ----------------
        def agg_window(w, tbl, row, nfeat, nh, adst_ap, ag_inst):
            """Aggregate one 128-dst window. Returns S psum tile
            [P, nfeat+nh] (features | ex-sums)."""
            width = nfeat + nh
            ps_S = ps_acc.tile([P, 774], F32, tag="acc")
            dT = edgep.tile([P, K], F32, tag="dlocT")
            nc.sync.dma_start(dT[:], dlocTd[w, :, :])
            # split gather (SBUF pressure)
            parts = []
            for gi, (g0, gn) in enumerate(((0, G1), (G1, G2))):
                xpg = gat.tile([P, G1 * row], F32, tag=f"xpg{gi}")
                idxt = edgep.tile([128, K * 8], I16, tag=f"idx{gi}")
                nc.sync.dma_start(idxt[:, 0:gn * 8],
                                  idx16d[w, :, g0 * 8:(g0 + gn) * 8])
                g = nc.gpsimd.dma_gather(
                    xpg[:].rearrange("p (k e) -> p k e", e=row)[:, 0:gn, :],
                    tbl[:], idxt[:, 0:gn * 8], gn * P, gn * P, row,
                    single_packet=False)
                tile.add_dep_helper(g.ins, ag_inst.ins, reason="gather after AG")
                parts.append(xpg)
            for t in range(K):
                gi, tl = (0, t) if t < G1 else (1, t - G1)
                xpg3 = parts[gi][:].rearrange("p (k e) -> p k e", e=row)
                # eqP[e,j] = (dstloc[e] == j)
                eqP = edgep.tile([P, P], F32, tag="eqP")
                nc.vector.tensor_tensor(
                    out=eqP[:], in0=iotaP_sb[:],
                    in1=dT[:, t:t + 1].to_broadcast([P, P]), op=OP.is_equal)
                # a_dst per edge: transpose(eqP) then matmul with window adst
                ps_T = ps_sm.tile([P, P], F32, tag="eqT")
                nc.tensor.transpose(out=ps_T[:], in_=eqP[:], identity=ident[:])
                eqT = edgep.tile([P, P], F32, tag="eqTs")
                nc.vector.tensor_copy(out=eqT[:], in_=ps_T[:])
                ps_a = ps_sm.tile([P, nh], F32, tag="adst")
                nc.tensor.matmul(out=ps_a[:], lhsT=eqT[:], rhs=adst_ap,
                                 start=True, stop=True)
                # ex = exp(lrelu(a_src + a_dst))
                rhs = edgep.tile([P, 774], F32, tag="rhs")
                al = edgep.tile([P, nh], F32, tag="al")
                nc.vector.tensor_tensor(
                    out=al[:], in0=xpg3[:, tl, nfeat:width], in1=ps_a[:],
                    op=OP.add)
                al2 = edgep.tile([P, nh], F32, tag="al2")
                nc.vector.tensor_scalar_mul(out=al2[:], in0=al[:],
                                            scalar1=NEG)
                nc.vector.tensor_tensor(out=al2[:], in0=al[:], in1=al2[:],
                                        op=OP.max)
                nc.scalar.activation(out=rhs[:, nfeat:width], in_=al2[:],
                                     func=AF.Exp)
                # rhs features = ex_h * gathered features
                for h in range(nh):
                    hs = slice(h * HID, (h + 1) * HID)
                    sc = rhs[:, nfeat + h:nfeat + h + 1]
                    if h == 2:
                        nc.vector.tensor_scalar_mul(
                            out=rhs[:, hs], in0=xpg3[:, tl, hs], scalar1=sc)
                    else:
                        nc.scalar.activation(out=rhs[:, hs],
                                             in_=xpg3[:, tl, hs],
                                             func=AF.Copy, scale=sc)
                st, sp = (t == 0), (t == K - 1)
                if width > 512:
                    nc.tensor.matmul(out=ps_S[:, 0:512], lhsT=eqP[:],
                                     rhs=rhs[:, 0:512], start=st, stop=sp)
                    nc.tensor.matmul(out=ps_S[:, 512:width], lhsT=eqP[:],
                                     rhs=rhs[:, 512:width], start=st, stop=sp)
                else:
                    nc.tensor.matmul(out=ps_S[:, 0:width], lhsT=eqP[:],
                                     rhs=rhs[:, 0:width], start=st, stop=sp)
            return ps_S

        # ---- phases 2+3: layer-1 aggregation, epilogue, xp2ext
        chunk2_writes = []
        for w in range(NWL if PH >= 3 else 0):
            ps_S = agg_window(w, table1, ROW1, 768, H1,
                              adst1_sb[:, w * H1:(w + 1) * H1], ag1)
            dm = edgep.tile([P, H1], F32, tag="dm")
            nc.vector.tensor_scalar_max(out=dm[:], in0=ps_S[:, 768:771],
                                        scalar1=1e-30)
            rc = edgep.tile([P, H1], F32, tag="rc")
            nc.vector.reciprocal(out=rc[:], in_=dm[:])
            y = stage.tile([P, 768], F32, tag="y")
            for h in range(H1):
                hs = slice(h * HID, (h + 1) * HID)
                nc.scalar.activation(out=y[:, hs], in_=ps_S[:, hs],
                                     func=AF.Copy, scale=rc[:, h:h + 1])
            nc.vector.tensor_add(out=y[:], in0=y[:], in1=b1_sb[:])
            # elu(y) = max(y, exp(min(y, 0)) - 1)
            t1 = stage.tile([P, 768], F32, tag="t1")
            nc.vector.tensor_scalar_min(out=t1[:], in0=y[:], scalar1=0.0)
            t2 = stage.tile([P, 768], F32, tag="t2")
            nc.scalar.activation(out=t2[:], in_=t1[:], func=AF.Exp)
            nc.vector.tensor_scalar_add(out=t2[:], in0=t2[:], scalar1=-1.0)
            hsb = stage.tile([P, 768], F32, tag="h")
            nc.vector.tensor_tensor(out=hsb[:], in0=y[:], in1=t2[:],
                                    op=OP.max)
            # hT + xp2ext
            hT = stage.tile([P, 768], F32, tag="hT")
            for c in range(NK1):
                cs = slice(c * P, (c + 1) * P)
                ps_t = ps_epi.tile([P, P], F32, tag="tr")
                nc.tensor.transpose(out=ps_t[:], in_=hsb[:, cs],
                                    identity=ident[:])
                nc.vector.tensor_copy(out=hT[:, cs], in_=ps_t[:])
            ps_x2 = ps_epi.tile([P, 258], F32, tag="x2")
            for c in range(NK1):
                nc.tensor.matmul(out=ps_x2[:], lhsT=hT[:, c * P:(c + 1) * P],
                                 rhs=w2e[c][:], start=(c == 0),
                                 stop=(c == NK1 - 1))
            xp2_sb = stage.tile([P, 257], F32, tag="xp2")
            nc.vector.tensor_copy(out=xp2_sb[:], in_=ps_x2[:, 0:257])
            nc.scalar.activation(out=a2dst_sb[:, w:w + 1],
                                 in_=ps_x2[:, 257:258], func=AF.Copy)
            rows = min(C - w * P, P)
            d = nc.sync.dma_start(out=chunk2[w * P:w * P + rows, 0:257],
                                  in_=xp2_sb[:rows, :])
            chunk2_writes.append(d)

        if PH >= 4:
            ag2 = nc.gpsimd.collective_compute(
                "AllGather", OP.bypass, replica_groups=GRPS,
                ins=[chunk2[:]], outs=[table2[:]])
            for d in chunk2_writes:
                tile.add_dep_helper(ag2.ins, d.ins,
                                    reason="chunk2 before AG2")

        # ---- phase 4: layer-2 aggregation -> out
        for w in range(NWL if PH >= 5 else 0):
            ps_S = agg_window(w, table2, ROW2, 256, 1,
                              a2dst_sb[:, w:w + 1], ag2)
            dm = edgep.tile([P, 1], F32, tag="dm2")
            nc.vector.tensor_scalar_max(out=dm[:], in0=ps_S[:, 256:257],
                                        scalar1=1e-30)
            rc = edgep.tile([P, 1], F32, tag="rc2")
            nc.vector.reciprocal(out=rc[:], in_=dm[:])
            o = stage.tile([P, HID], F32, tag="o")
            nc.scalar.activation(out=o[:], in_=ps_S[:, 0:256], func=AF.Copy,
                                 scale=rc[:])
            nc.vector.tensor_add(out=o[:], in0=o[:], in1=b2_sb[:])
            nc.sync.dma_start(out=outd[w * P:(w + 1) * P, :], in_=o[:])

    nc.compile()
    return nc


_prog_cache = {}
LAST_EXEC_NS = None
LAST_TRACE = None


def kernel(**inputs):
    x = np.ascontiguousarray(np.asarray(inputs["x"], np.float32))
    ei = np.asarray(inputs["edge_index"]).astype(np.int64)
    W1 = np.asarray(inputs["W1"], np.float32)
    as1 = np.asarray(inputs["as1"], np.float32)
    ad1 = np.asarray(inputs["ad1"], np.float32)
    b1 = np.asarray(inputs["b1"], np.float32)
    W2 = np.asarray(inputs["W2"], np.float32)
    as2 = np.asarray(inputs["as2"], np.float32)
    ad2 = np.asarray(inputs["ad2"], np.float32)
    b2 = np.asarray(inputs["b2"], np.float32)

    idx16, dstlocT, _dstloc, K = prep_edges(ei)

    if K not in _prog_cache:
        _prog_cache[K] = build_program(K)
    nc = _prog_cache[K]

    def bc(v, width):  # replicate a row vector across 128 partitions
        return np.ascontiguousarray(
            np.broadcast_to(v.reshape(1, width), (P, width)).astype(np.float32))

    shared = {
        "W1": np.ascontiguousarray(W1),
        "W2": np.ascontiguousarray(W2),
        "b1bc": bc(b1, H1 * HID),
        "b2bc": bc(b2, HID),
        "as1bc": bc(as1.reshape(-1), H1 * HID),
        "ad1bc": bc(ad1.reshape(-1), H1 * HID),
        "as2bc": bc(as2.reshape(-1), HID),
        "ad2bc": bc(ad2.reshape(-1), HID),
        "iotaP": np.ascontiguousarray(
            np.broadcast_to(np.arange(P, dtype=np.float32), (P, P))),
    }
    in_maps = []
    for r in range(R):
        xT = np.zeros((IND, CP), np.float32)
        xT[:, :C] = x[r * C:(r + 1) * C].T
        in_maps.append({
            **shared,
            "xT": xT,
            "idx16": np.ascontiguousarray(idx16[r]),
            "dstlocT": np.ascontiguousarray(dstlocT[r]),
        })

    trace = bool(int(os.environ.get("GAT_TRACE", "0")))
    res = run_bass_kernel_spmd(nc, in_maps, list(range(R)), trace=trace)
    global LAST_EXEC_NS, LAST_TRACE
    LAST_EXEC_NS = res.exec_time_ns
    LAST_TRACE = (res.instructions_and_trace[1]
                  if res.instructions_and_trace else None)
    if trace:
        print("exec_time_ns:", res.exec_time_ns,
              "mean:", res.mean_exec_time_ns, "trace:", LAST_TRACE)
        if res.per_core_scope_times:
            for scope, d in sorted(res.per_core_scope_times.items()):
                print("  scope", scope, d)
    out = np.concatenate(
        [res.results[r]["out_local"][:C] for r in range(R)], axis=0)
    return np.ascontiguousarray(out.astype(np.float32))


if __name__ == "__main__":
    import reference

    inputs = reference.setup_inputs()
    inputs = {k: np.asarray(v) for k, v in inputs.items()}
    out = kernel(**inputs)
    print("kernel output", out.shape, out.dtype)
